# revision 1
# baseline (speedup 1.0000x reference)
"""MultiHead KAN Attention (B=2, L=1024, E=512, H=8, G=32 cubic B-splines)
for Trainium2, distributed over 8 NeuronCores.

Sharding: core c = b*4 + l2*2 + pi  --  batch x L-half x out-half.
Each core computes KAN projections Q/K/V for its 512 rows and its 256
output columns (4 heads) via one dense 18432-contraction float32r matmul
whose rows are [35 B-spline-basis slots x 512 channels] + [512 SiLU base
rows]; the basis rows are generated on-chip (exact cardinal cubic
B-spline) by custom DVE ops and ACT abs/relu; attention (no softmax) is
K^T V with a 2-core AllReduce, then Q @ (K^T V).

Self-contained: registers its custom DVE ops and patches the Tile tail
drain (walrus build here allows only 1 sync-wait per CTRL instruction).
"""
import numpy as np

# ---------------------------------------------------------------- patches
import concourse.tile as _tile_mod
from concourse.vector_clock import ScopedClock, VectorClock
from concourse.tile_scheduler import N_PROCS


def _patched_drain_and_barrier(self, tick_clock, wait_clock):
    gc = tick_clock.global_clock
    procs = [p for p in range(N_PROCS) if gc[p] > 0]
    for i in range(len(procs)):
        chunk = {procs[i]}
        vc = VectorClock([gc[p] if p in chunk else 0 for p in range(N_PROCS)])
        nop = self.nc.sync.nop(nofuse=True)
        wait_clock.add_sem_waits(nop.ins, ScopedClock({None: vc}))
    self.nc.sync.drain()  # waits covered by the SP nops above (program order)
    self.nc.all_engine_barrier()
    assert self.sems is not None
    popped = self.nc._tile_sem_poison_stack.pop()
    assert popped is self._sem_poison
    self.nc.clear_and_free_semaphores(list(self.sems.allocated().values()))
    self.nc.all_engine_barrier()


_tile_mod.TileContext._drain_and_barrier = _patched_drain_and_barrier

# ------------------------------------------------------- custom DVE ops
from concourse.dve_spec import (
    Spec, lower, Src0, Src1, C0, C2, Zero, One, maxx, Bin, AluOp, _has_src1,
)
import concourse.dve_ops as dve_ops
from concourse.dve_ops import DveOp
from concourse.dve_uop import DveOpSpec


def _mk_op(name, spec):
    for o in dve_ops.OPS:
        if o.name == name:
            return o
    row = dve_ops._CUSTOM_DVE_ROW_BASE + len(dve_ops.OPS)
    assert row < 0x20
    shas = {}
    for ver in ("v3", "v4"):
        res = DveOpSpec(name=name, opcode=row, uops=lower(spec, ver=ver),
                        rd1_en=_has_src1(spec))
        shas[ver] = res.sha(ver)
    op = DveOp(name, spec, subdim=False, uops_sha=shas)
    dve_ops.OPS.append(op)
    dve_ops._SUB_OPCODE_FOR_NAME[name] = row
    dve_ops.CUSTOM_DVE_SPECS[name] = spec
    return op


def _adiff(a, b):
    return Bin(AluOp.ABSOLUTE_DIFF, a, b)


_K2 = One + One

# 6*B3 via v = |z - tau|:  B6 = relu(2-v)^3 - 4*relu(1-v)^3
_v = _adiff(Src0, C0)
_a = maxx(_K2 - _v, Zero)
BSPL3A = _mk_op("BSPL3A", Spec(
    body=_a * _a * _a,
    reference=lambda in0, s0, **k: np.maximum(2.0 - np.abs(in0 - s0), 0.0) ** 3,
))

_v2 = _adiff(Src0, C0)
_b = maxx(One - _v2, Zero)
_b3 = _b * _b * _b
BSPL3B = _mk_op("BSPL3B", Spec(
    body=Src1 - _b3 * C2,
    reference=lambda in0, in1, s0, imm2, **k:
        in1 - imm2 * np.maximum(1.0 - np.abs(in0 - s0), 0.0) ** 3,
))

_e = Src0 - One
_r = maxx(_e, Zero)
_r3 = _r * _r * _r
_s3 = Src0 * Src0 * Src0
BSPL3S = _mk_op("BSPL3S", Spec(
    body=_s3 - _r3 * C2,
    reference=lambda in0, imm2, **k:
        in0 ** 3 - imm2 * np.maximum(in0 - 1.0, 0.0) ** 3,
))

# ------------------------------------------------------------ the kernel
import concourse.bass as bass
import concourse.bacc as bacc
import concourse.tile as tile
import concourse.mybir as mybir
from concourse.masks import make_identity
from concourse.tile_rust import add_dep_helper

F32 = mybir.dt.float32
F32R = mybir.dt.float32r

R = 512
E = 512
NB = 35
KD_TILES = 144
O = 256
HLOC = 4
DH = 64
GRID_H = 6.0 / 32.0
INV_H = 1.0 / GRID_H
XLO, XHI = -3.0, 3.0 - 1e-4

CFG = dict(
    p3_gs=frozenset(g for g in range(NB) if g % 3 != 2),
    w_bufs=8,
    at_bufs=5,
    a3_bufs=2,
    u_bufs=2,
)


def _build_kernel(cfg=None):
    cfg = {**CFG, **(cfg or {})}
    nc = bacc.Bacc()

    xT = {p: nc.dram_tensor(f"x{p}", [128, 4 * R], F32, kind="ExternalInput").ap()
          for p in "qkv"}
    W = {p: nc.dram_tensor(f"w{p}", [KD_TILES * 128, O], F32R, kind="ExternalInput").ap()
         for p in "qkv"}
    actbias = nc.dram_tensor("actbias", [128, NB + 1], F32, kind="ExternalInput").ap()
    out_d = nc.dram_tensor("out", [R, O], F32, kind="ExternalOutput").ap()

    cc_in = nc.dram_tensor("cc_in", [DH, O], F32)
    cc_out = nc.dram_tensor("cc_out", [DH, O], F32)
    groups = [[0, 2], [1, 3], [4, 6], [5, 7]]

    with tile.TileContext(nc, num_cores=8) as tc:
        with (
            tc.tile_pool(name="persist", bufs=1) as pp,
            tc.tile_pool(name="stage", bufs=1) as sp,
            tc.tile_pool(name="wpool", bufs=cfg["w_bufs"]) as wp,
            tc.tile_pool(name="atpool", bufs=cfg["at_bufs"]) as atp,
            tc.tile_pool(name="mid", bufs=max(cfg["a3_bufs"], cfg["u_bufs"])) as mp,
            tc.tile_pool(name="psum", bufs=1, space="PSUM") as psp,
            tc.tile_pool(name="psum_small", bufs=2, space="PSUM") as pss,
        ):
            ab = pp.tile([128, NB + 1], F32, tag="ab")
            nc.sync.dma_start(ab[:], actbias[:])
            ident = pp.tile([128, 128], F32, tag="ident")
            make_identity(nc, ident[:])

            xt_out = {}
            for p in "qkv":
                xt_out[p] = [pp.tile([128, R], F32, tag=f"o_{p}{m}", name=f"o_{p}{m}")
                             for m in range(2)]

            def projection(p):
                xin = sp.tile([128, 4 * R], F32, tag="xin", name=f"xin_{p}")
                nc.sync.dma_start(xin[:], xT[p][:])
                sl = sp.tile([128, 4 * R], F32R, tag="sl", name=f"sl_{p}")
                nc.scalar.activation(sl[:], xin[:], mybir.ActivationFunctionType.Silu)
                zt = sp.tile([128, 4 * R], F32, tag="zt", name=f"zt_{p}")
                nc.vector.tensor_scalar(zt[:], xin[:], XHI, XLO,
                                        mybir.AluOpType.min, mybir.AluOpType.max)
                nc.vector.tensor_scalar(zt[:], zt[:], INV_H, XLO * -INV_H,
                                        mybir.AluOpType.mult, mybir.AluOpType.add)

                ps = [psp.tile([128, R], F32, tag=f"ps{m}", name=f"ps_{p}{m}")
                      for m in range(2)]

                cur_at = None
                for kt in range(KD_TILES):
                    if kt < 140:
                        g, eb = kt // 4, kt % 4
                        if eb == 0:
                            tau = float(g - 1)
                            if g in cfg["p3_gs"]:
                                u = mp.tile([128, 4 * R], F32, tag="u", name=f"u_{p}{g}")
                                nc.scalar.activation(
                                    u[:], zt[:], mybir.ActivationFunctionType.Abs,
                                    bias=ab[:, g:g + 1], scale=1.0)
                                sg = mp.tile([128, 4 * R], F32, tag="sg", name=f"sg_{p}{g}")
                                nc.scalar.activation(
                                    sg[:], u[:], mybir.ActivationFunctionType.Relu,
                                    bias=ab[:, NB:NB + 1], scale=-1.0)
                                at = atp.tile([128, 4 * R], F32R, tag="at", name=f"at_{p}{g}")
                                nc.vector._custom_dve(BSPL3S, out=at[:], in0=sg[:], imm2=4.0)
                            else:
                                a3 = mp.tile([128, 4 * R], F32, tag="a3", name=f"a3_{p}{g}")
                                nc.vector._custom_dve(BSPL3A, out=a3[:], in0=zt[:], s0=tau)
                                at = atp.tile([128, 4 * R], F32R, tag="at", name=f"at_{p}{g}")
                                nc.vector._custom_dve(BSPL3B, out=at[:], in0=zt[:],
                                                      in1=a3[:], s0=tau, imm2=4.0)
                            cur_at = at
                        operand = cur_at[:, eb * R:(eb + 1) * R]
                    else:
                        eb = kt - 140
                        operand = sl[:, eb * R:(eb + 1) * R]

                    wt = wp.tile([128, O], F32R, tag="wt", name=f"wt_{p}{kt}")
                    nc.sync.dma_start(wt[:], W[p][kt * 128:(kt + 1) * 128, :])
                    for m in range(2):
                        nc.tensor.matmul(
                            ps[m][:], wt[:, m * 128:(m + 1) * 128], operand,
                            start=(kt == 0), stop=(kt == KD_TILES - 1))

                for m in range(2):
                    nc.vector.tensor_copy(xt_out[p][m][:], ps[m][:])

            def transpose_to_bl(p):
                res = []
                for blk in range(4):
                    t = pp.tile([128, O], F32, tag=f"t_{p}{blk}", name=f"t_{p}{blk}")
                    for m in range(2):
                        tp = pss.tile([128, 128], F32, tag="tp", name=f"tp_{p}{blk}{m}")
                        nc.tensor.transpose(
                            tp[:], xt_out[p][m][:, blk * 128:(blk + 1) * 128], ident[:])
                        nc.vector.tensor_copy(t[:, m * 128:(m + 1) * 128], tp[:])
                    res.append(t)
                return res

            projection("k")
            projection("v")
            K_sb = transpose_to_bl("k")
            V_sb = transpose_to_bl("v")

            ktv_ps = pss.tile([DH, O], F32, tag="ktv")
            for h in range(HLOC):
                for blk in range(4):
                    nc.tensor.matmul(
                        ktv_ps[:, h * DH:(h + 1) * DH],
                        K_sb[blk][:, h * DH:(h + 1) * DH],
                        V_sb[blk][:, h * DH:(h + 1) * DH],
                        start=(blk == 0), stop=(blk == 3))
            ktv_sb = pp.tile([DH, O], F32, tag="ktv_sb")
            nc.vector.tensor_copy(ktv_sb[:], ktv_ps[:])

            d1 = nc.gpsimd.dma_start(cc_in.ap(), ktv_sb[:])
            cc = nc.gpsimd.collective_compute(
                "AllReduce", mybir.AluOpType.add, replica_groups=groups,
                ins=[cc_in.ap().opt()], outs=[cc_out.ap().opt()])
            add_dep_helper(cc.ins, d1.ins, reason="allreduce waits on cc_in write")
            ktv_red = pp.tile([128, O], F32, tag="ktv_red")
            d2 = nc.gpsimd.dma_start(ktv_red[0:DH, :], cc_out.ap())
            add_dep_helper(d2.ins, cc.ins, reason="readback waits on allreduce")
            d3 = nc.gpsimd.dma_start(ktv_red[DH:2 * DH, :], cc_out.ap())
            add_dep_helper(d3.ins, cc.ins, reason="readback2 waits on allreduce")

            projection("q")

            out_sb = [pp.tile([128, O], F32, tag=f"os{blk}", name=f"os{blk}")
                      for blk in range(4)]
            for blk in range(4):
                for h in range(HLOC):
                    po = pss.tile([128, DH], F32, tag="po", name=f"po_{blk}{h}")
                    nc.tensor.matmul(
                        po[:],
                        xt_out["q"][h // 2][(h % 2) * DH:(h % 2 + 1) * DH,
                                            blk * 128:(blk + 1) * 128],
                        ktv_red[(h % 2) * DH:(h % 2 + 1) * DH, h * DH:(h + 1) * DH],
                        start=True, stop=True)
                    nc.vector.tensor_copy(out_sb[blk][:, h * DH:(h + 1) * DH], po[:])
                nc.sync.dma_start(out_d[blk * 128:(blk + 1) * 128, :], out_sb[blk][:])

    nc.compile()
    return nc


def _host_prep(inputs):
    q, k, v = inputs["query"], inputs["key"], inputs["value"]
    Ws = {}
    for p, (wb, wsp) in (("q", ("wq_base", "wq_sp")), ("k", ("wk_base", "wk_sp")),
                         ("v", ("wv_base", "wv_sp"))):
        wsp_a = np.ascontiguousarray(
            np.transpose(np.asarray(inputs[wsp], np.float32), (1, 0, 2))
            .reshape(NB * E, E) / 6.0).astype(np.float32)
        Ws[p] = np.concatenate([wsp_a, np.asarray(inputs[wb], np.float32)], axis=0)

    ab = np.zeros((128, NB + 1), np.float32)
    for g in range(NB):
        ab[:, g] = float(1 - g)
    ab[:, NB] = 2.0

    def xt_of(x, b, l2):
        s = np.asarray(x[b, l2 * R:(l2 + 1) * R, :], np.float32)
        t = s.T.reshape(4, 128, R)
        return np.ascontiguousarray(t.transpose(1, 0, 2).reshape(128, 4 * R))

    in_maps = []
    for c in range(8):
        b, l2, pi = c >> 2, (c >> 1) & 1, c & 1
        m = {"xq": xt_of(q, b, l2), "xk": xt_of(k, b, l2), "xv": xt_of(v, b, l2),
             "actbias": ab}
        for p in "qkv":
            m[f"w{p}"] = np.ascontiguousarray(Ws[p][:, pi * O:(pi + 1) * O])
        in_maps.append(m)
    return in_maps


_NC_CACHE = {}


def kernel(**inputs):
    from concourse.bass_utils import run_bass_kernel_spmd
    if "nc" not in _NC_CACHE:
        _NC_CACHE["nc"] = _build_kernel()
    nc = _NC_CACHE["nc"]
    in_maps = _host_prep(inputs)
    res = run_bass_kernel_spmd(nc, in_maps, core_ids=list(range(8)))
    out = np.zeros((2, 1024, 512), np.float32)
    for c in range(8):
        b, l2, pi = c >> 2, (c >> 1) & 1, c & 1
        out[b, l2 * R:(l2 + 1) * R, pi * O:(pi + 1) * O] = res.results[c]["out"]
    return out


# revision 3
# speedup vs baseline: 36503.0546x; 36503.0546x over previous
"""MultiHead KAN Attention (B=2, L=1024, E=512, H=8, G=32 cubic B-splines)
for Trainium2, distributed over 8 NeuronCores.

Sharding: core c = b*4 + l2*2 + pi  --  batch x L-half x out-half.
Each core computes KAN projections Q/K/V for its 512 rows and its 256
output columns (4 heads) via one dense 18432-contraction float32r matmul
whose rows are [35 B-spline-basis slots x 512 channels] + [512 SiLU base
rows]; the basis rows are generated on-chip (exact cardinal cubic
B-spline) by custom DVE ops and ACT abs/relu; attention (no softmax) is
K^T V with a 2-core AllReduce, then Q @ (K^T V).

Self-contained: registers its custom DVE ops and patches the Tile tail
drain (walrus build here allows only 1 sync-wait per CTRL instruction).
"""
import numpy as np

# ---------------------------------------------------------------- patches
import concourse.tile as _tile_mod
from concourse.vector_clock import ScopedClock, VectorClock
from concourse.tile_scheduler import N_PROCS


def _patched_drain_and_barrier(self, tick_clock, wait_clock):
    gc = tick_clock.global_clock
    procs = [p for p in range(N_PROCS) if gc[p] > 0]
    for i in range(len(procs)):
        chunk = {procs[i]}
        vc = VectorClock([gc[p] if p in chunk else 0 for p in range(N_PROCS)])
        nop = self.nc.sync.nop(nofuse=True)
        wait_clock.add_sem_waits(nop.ins, ScopedClock({None: vc}))
    self.nc.sync.drain()  # waits covered by the SP nops above (program order)
    self.nc.all_engine_barrier()
    assert self.sems is not None
    popped = self.nc._tile_sem_poison_stack.pop()
    assert popped is self._sem_poison
    self.nc.clear_and_free_semaphores(list(self.sems.allocated().values()))
    self.nc.all_engine_barrier()


_tile_mod.TileContext._drain_and_barrier = _patched_drain_and_barrier

# ------------------------------------------------------- custom DVE ops
from concourse.dve_spec import (
    Spec, lower, Src0, Src1, C0, C2, Zero, One, maxx, Bin, AluOp, _has_src1,
)
import concourse.dve_ops as dve_ops
from concourse.dve_ops import DveOp
from concourse.dve_uop import DveOpSpec


def _mk_op(name, spec):
    for o in dve_ops.OPS:
        if o.name == name:
            return o
    row = dve_ops._CUSTOM_DVE_ROW_BASE + len(dve_ops.OPS)
    assert row < 0x20
    shas = {}
    for ver in ("v3", "v4"):
        res = DveOpSpec(name=name, opcode=row, uops=lower(spec, ver=ver),
                        rd1_en=_has_src1(spec))
        shas[ver] = res.sha(ver)
    op = DveOp(name, spec, subdim=False, uops_sha=shas)
    dve_ops.OPS.append(op)
    dve_ops._SUB_OPCODE_FOR_NAME[name] = row
    dve_ops.CUSTOM_DVE_SPECS[name] = spec
    return op


def _adiff(a, b):
    return Bin(AluOp.ABSOLUTE_DIFF, a, b)


_K2 = One + One

# 6*B3 via v = |z - tau|:  B6 = relu(2-v)^3 - 4*relu(1-v)^3
_v = _adiff(Src0, C0)
_a = maxx(_K2 - _v, Zero)
BSPL3A = _mk_op("BSPL3A", Spec(
    body=_a * _a * _a,
    reference=lambda in0, s0, **k: np.maximum(2.0 - np.abs(in0 - s0), 0.0) ** 3,
))

_v2 = _adiff(Src0, C0)
_b = maxx(One - _v2, Zero)
_b3 = _b * _b * _b
BSPL3B = _mk_op("BSPL3B", Spec(
    body=Src1 - _b3 * C2,
    reference=lambda in0, in1, s0, imm2, **k:
        in1 - imm2 * np.maximum(1.0 - np.abs(in0 - s0), 0.0) ** 3,
))

_e = Src0 - One
_r = maxx(_e, Zero)
_r3 = _r * _r * _r
_s3 = Src0 * Src0 * Src0
BSPL3S = _mk_op("BSPL3S", Spec(
    body=_s3 - _r3 * C2,
    reference=lambda in0, imm2, **k:
        in0 ** 3 - imm2 * np.maximum(in0 - 1.0, 0.0) ** 3,
))


# ----------------------------------------------- forged ACT table (B-spline)
import json as _json, struct as _struct, shutil as _shutil, os as _os, tempfile as _tempfile

_ACT_SRC = "/nix/store/z022hj2nvbm3nwdizlisq4ylc0y7rd6q-python3-3.13.14-env/lib/python3.13/site-packages/neuronxcc/pwp/pwp_bin_trainium"


def _forge_act_tables():
    """Rewrite `tanh` in silu_and_others as F(v)=relu(2-|x|)^3-4*relu(1-|x|)^3
    (= 6*B3 cardinal cubic B-spline), exact piecewise-cubic buckets, even
    symmetry. Tables ship inside the NEFF, so a compile-time override is all
    that is needed."""
    dst = _os.path.join(_tempfile.gettempdir(), "kan_act_forged")
    marker = _os.path.join(dst, ".forged_b6_v1")
    if not _os.path.exists(marker):
        if _os.path.exists(dst):
            _shutil.rmtree(dst)
        _os.makedirs(dst)
        for f in _os.listdir(_ACT_SRC):
            _shutil.copy(_os.path.join(_ACT_SRC, f), _os.path.join(dst, f))
        sj = _json.load(open(f"{dst}/silu_and_others.json"))
        bkt = bytearray(open(f"{dst}/silu_and_others_bkt.bin", "rb").read())

        def taylor(x0):
            if x0 < 1.0:
                return (3*x0**3 - 6*x0**2 + 4, 9*x0**2 - 12*x0, 9*x0 - 6, 3.0)
            if x0 < 2.0:
                w = 2.0 - x0
                return (w**3, -3*w**2, 3*w, -1.0)
            return (0.0, 0.0, 0.0, 0.0)

        def put(i, d0, d1, d2, d3, x0):
            _struct.pack_into("<8f", bkt, i*32, d0, d1, d2, d3, x0, 0, 0, 0)

        e2b = sj["func_exp_to_bkt_start_idx"]["tanh"]
        exps = sorted(int(e) for e in e2b)
        starts = {int(e): v[0] for e, v in e2b.items()}
        pm = [p for p in sj["profile_meta_data"] if p["func_name"].startswith("tanh")][0]
        specials = (pm["pos_small_signal_pwl_control"], pm["neg_small_signal_pwl_control"],
                    pm["pos_large_signal_pwl_control"], pm["neg_large_signal_pwl_control"])
        end = min(specials)
        for j, e in enumerate(exps):
            s = starts[e]
            s_next = starts[exps[j+1]] if j+1 < len(exps) else end
            cnt = s_next - s
            lo = 2.0 ** e
            width = lo / cnt
            for i in range(cnt):
                x0 = lo + (i + 0.5) * width
                put(s + i, *taylor(x0), x0)
        put(specials[0], 4.0, 0, 0, 0, 0.0)
        put(specials[1], 4.0, 0, 0, 0, 0.0)
        put(specials[2], 0.0, 0, 0, 0, 0.0)
        put(specials[3], 0.0, 0, 0, 0, 0.0)
        open(f"{dst}/silu_and_others_bkt.bin", "wb").write(bytes(bkt))
        pm["sym_invert_sign_point"] = 0
        pm["symmetry_opt_en"] = 1
        pm["fzero_result"] = _struct.unpack("<I", _struct.pack("<f", 4.0))[0]
        pm["fpinf_result"] = 0
        pm["fninf_result"] = 0
        _json.dump(sj, open(f"{dst}/silu_and_others.json", "w"))
        open(marker, "w").write("ok")
    _os.environ["BASS_ACT_ROOT_JSON_PATH"] = f"{dst}/act_info.json"


_forge_act_tables()

# ------------------------------------------------------------ the kernel
import concourse.bass as bass
import concourse.bacc as bacc
import concourse.tile as tile
import concourse.mybir as mybir
from concourse.masks import make_identity
from concourse.tile_rust import add_dep_helper

F32 = mybir.dt.float32
F32R = mybir.dt.float32r

R = 512
E = 512
NB = 35
KD_TILES = 144
O = 256
HLOC = 4
DH = 64
GRID_H = 6.0 / 32.0
INV_H = 1.0 / GRID_H
XLO, XHI = -3.0, 3.0 - 1e-4

CFG = dict(
    p3_gs=frozenset(g for g in range(NB) if g % 3 != 2),  # table-path g's
    w_bufs=8,
    at_bufs=8,
    a3_bufs=3,
    u_bufs=2,
)


def _build_kernel(cfg=None):
    cfg = {**CFG, **(cfg or {})}
    nc = bacc.Bacc()

    xT = {p: nc.dram_tensor(f"x{p}", [128, 4 * R], F32, kind="ExternalInput").ap()
          for p in "qkv"}
    W = {p: nc.dram_tensor(f"w{p}", [KD_TILES * 128, O], F32R, kind="ExternalInput").ap()
         for p in "qkv"}
    actbias = nc.dram_tensor("actbias", [128, NB + 1], F32, kind="ExternalInput").ap()
    out_d = nc.dram_tensor("out", [R, O], F32, kind="ExternalOutput").ap()

    cc_in = nc.dram_tensor("cc_in", [DH, O], F32)
    cc_out = nc.dram_tensor("cc_out", [DH, O], F32)
    groups = [[0, 2], [1, 3], [4, 6], [5, 7]]

    with tile.TileContext(nc, num_cores=1 if cfg.get("fake_cc") else 8) as tc:
        with (
            tc.tile_pool(name="persist", bufs=1) as pp,
            tc.tile_pool(name="stage", bufs=1) as sp,
            tc.tile_pool(name="wpool", bufs=cfg["w_bufs"]) as wp,
            tc.tile_pool(name="atpool", bufs=cfg["at_bufs"]) as atp,
            tc.tile_pool(name="mid", bufs=max(cfg["a3_bufs"], cfg["u_bufs"])) as mp,
            tc.tile_pool(name="psum", bufs=1, space="PSUM") as psp,
            tc.tile_pool(name="psum_small", bufs=2, space="PSUM") as pss,
        ):
            ab = pp.tile([128, NB + 1], F32, tag="ab")
            nc.sync.dma_start(ab[:], actbias[:])
            ident = pp.tile([128, 128], F32, tag="ident")
            make_identity(nc, ident[:])

            xt_out = {}
            for p in "qkv":
                xt_out[p] = [pp.tile([128, R], F32, tag=f"o_{p}{m}", name=f"o_{p}{m}")
                             for m in range(2)]

            def projection(p):
                xin = sp.tile([128, 4 * R], F32, tag="xin", name=f"xin_{p}")
                nc.sync.dma_start(xin[:], xT[p][:])
                sl = sp.tile([128, 4 * R], F32R, tag="sl", name=f"sl_{p}")
                nc.scalar.activation(sl[:], xin[:], mybir.ActivationFunctionType.Silu)
                zt = sp.tile([128, 4 * R], F32, tag="zt", name=f"zt_{p}")
                nc.vector.tensor_scalar(zt[:], xin[:], XHI, XLO,
                                        mybir.AluOpType.min, mybir.AluOpType.max)
                nc.vector.tensor_scalar(zt[:], zt[:], INV_H, XLO * -INV_H,
                                        mybir.AluOpType.mult, mybir.AluOpType.add)

                ps = [psp.tile([128, R], F32, tag=f"ps{m}", name=f"ps_{p}{m}")
                      for m in range(2)]

                # W viewed as [36 grp, 4 kt, 128 p, 256 o]; 4 k-tiles per
                # DMA so the PE pays one sem-wait per 8 accumulating matmuls
                Wv = W[p].rearrange("(grp four p) o -> grp p four o", four=4, p=128)
                for grp in range(36):
                    g = grp
                    if grp < 35:
                        tau = float(g - 1)
                        if g in cfg["p3_gs"]:
                            # forged table: one ACT op evaluates 6*B3(|z-tau|)
                            at = atp.tile([128, 4 * R], F32R, tag="at", name=f"at_{p}{g}")
                            nc.scalar.activation(
                                at[:], zt[:], mybir.ActivationFunctionType.Tanh,
                                bias=ab[:, g:g + 1], scale=1.0)
                        else:
                            a3 = mp.tile([128, 4 * R], F32, tag="a3", name=f"a3_{p}{g}")
                            nc.vector._custom_dve(BSPL3A, out=a3[:], in0=zt[:], s0=tau)
                            at = atp.tile([128, 4 * R], F32R, tag="at", name=f"at_{p}{g}")
                            nc.vector._custom_dve(BSPL3B, out=at[:], in0=zt[:],
                                                  in1=a3[:], s0=tau, imm2=4.0)
                        src_tile = at
                    else:
                        src_tile = sl
                    wt4 = wp.tile([128, 4, O], F32R, tag="wt", name=f"wt_{p}{grp}")
                    nc.sync.dma_start(wt4[:], Wv[grp])
                    for eb in range(4):
                        kt = grp * 4 + eb
                        operand = src_tile[:, eb * R:(eb + 1) * R]
                        for m in range(2):
                            nc.tensor.matmul(
                                ps[m][:], wt4[:, eb, m * 128:(m + 1) * 128], operand,
                                start=(kt == 0), stop=(kt == KD_TILES - 1))

                for m in range(2):
                    nc.vector.tensor_copy(xt_out[p][m][:], ps[m][:])

            def transpose_to_bl(p):
                res = []
                for blk in range(4):
                    t = pp.tile([128, O], F32, tag=f"t_{p}{blk}", name=f"t_{p}{blk}")
                    for m in range(2):
                        tp = pss.tile([128, 128], F32, tag="tp", name=f"tp_{p}{blk}{m}")
                        nc.tensor.transpose(
                            tp[:], xt_out[p][m][:, blk * 128:(blk + 1) * 128], ident[:])
                        nc.vector.tensor_copy(t[:, m * 128:(m + 1) * 128], tp[:])
                    res.append(t)
                return res

            projection("k")
            projection("v")
            K_sb = transpose_to_bl("k")
            V_sb = transpose_to_bl("v")

            ktv_ps = pss.tile([DH, O], F32, tag="ktv")
            for h in range(HLOC):
                for blk in range(4):
                    nc.tensor.matmul(
                        ktv_ps[:, h * DH:(h + 1) * DH],
                        K_sb[blk][:, h * DH:(h + 1) * DH],
                        V_sb[blk][:, h * DH:(h + 1) * DH],
                        start=(blk == 0), stop=(blk == 3))
            ktv_sb = pp.tile([DH, O], F32, tag="ktv_sb")
            nc.vector.tensor_copy(ktv_sb[:], ktv_ps[:])

            d1 = nc.gpsimd.dma_start(cc_in.ap(), ktv_sb[:])
            if cfg.get("fake_cc"):
                cc = nc.gpsimd.dma_start(cc_out.ap(), cc_in.ap())
            else:
                cc = nc.gpsimd.collective_compute(
                    "AllReduce", mybir.AluOpType.add, replica_groups=groups,
                    ins=[cc_in.ap().opt()], outs=[cc_out.ap().opt()])
            add_dep_helper(cc.ins, d1.ins, reason="allreduce waits on cc_in write")
            ktv_red = pp.tile([128, O], F32, tag="ktv_red")
            d2 = nc.gpsimd.dma_start(ktv_red[0:DH, :], cc_out.ap())
            add_dep_helper(d2.ins, cc.ins, reason="readback waits on allreduce")
            d3 = nc.gpsimd.dma_start(ktv_red[DH:2 * DH, :], cc_out.ap())
            add_dep_helper(d3.ins, cc.ins, reason="readback2 waits on allreduce")

            projection("q")

            out_sb = [pp.tile([128, O], F32, tag=f"os{blk}", name=f"os{blk}")
                      for blk in range(4)]
            for blk in range(4):
                for h in range(HLOC):
                    po = pss.tile([128, DH], F32, tag="po", name=f"po_{blk}{h}")
                    nc.tensor.matmul(
                        po[:],
                        xt_out["q"][h // 2][(h % 2) * DH:(h % 2 + 1) * DH,
                                            blk * 128:(blk + 1) * 128],
                        ktv_red[(h % 2) * DH:(h % 2 + 1) * DH, h * DH:(h + 1) * DH],
                        start=True, stop=True)
                    nc.vector.tensor_copy(out_sb[blk][:, h * DH:(h + 1) * DH], po[:])
                nc.sync.dma_start(out_d[blk * 128:(blk + 1) * 128, :], out_sb[blk][:])

    nc.compile()
    return nc


def _host_prep(inputs):
    q, k, v = inputs["query"], inputs["key"], inputs["value"]
    Ws = {}
    for p, (wb, wsp) in (("q", ("wq_base", "wq_sp")), ("k", ("wk_base", "wk_sp")),
                         ("v", ("wv_base", "wv_sp"))):
        wsp_a = np.ascontiguousarray(
            np.transpose(np.asarray(inputs[wsp], np.float32), (1, 0, 2))
            .reshape(NB * E, E) / 6.0).astype(np.float32)
        Ws[p] = np.concatenate([wsp_a, np.asarray(inputs[wb], np.float32)], axis=0)

    ab = np.zeros((128, NB + 1), np.float32)
    for g in range(NB):
        ab[:, g] = float(1 - g)
    ab[:, NB] = 2.0

    def xt_of(x, b, l2):
        s = np.asarray(x[b, l2 * R:(l2 + 1) * R, :], np.float32)
        t = s.T.reshape(4, 128, R)
        return np.ascontiguousarray(t.transpose(1, 0, 2).reshape(128, 4 * R))

    in_maps = []
    for c in range(8):
        b, l2, pi = c >> 2, (c >> 1) & 1, c & 1
        m = {"xq": xt_of(q, b, l2), "xk": xt_of(k, b, l2), "xv": xt_of(v, b, l2),
             "actbias": ab}
        for p in "qkv":
            m[f"w{p}"] = np.ascontiguousarray(Ws[p][:, pi * O:(pi + 1) * O])
        in_maps.append(m)
    return in_maps


_NC_CACHE = {}


def kernel(**inputs):
    from concourse.bass_utils import run_bass_kernel_spmd
    if "nc" not in _NC_CACHE:
        _NC_CACHE["nc"] = _build_kernel()
    nc = _NC_CACHE["nc"]
    in_maps = _host_prep(inputs)
    res = run_bass_kernel_spmd(nc, in_maps, core_ids=list(range(8)))
    out = np.zeros((2, 1024, 512), np.float32)
    for c in range(8):
        b, l2, pi = c >> 2, (c >> 1) & 1, c & 1
        out[b, l2 * R:(l2 + 1) * R, pi * O:(pi + 1) * O] = res.results[c]["out"]
    return out
